# revision 1
# baseline (speedup 1.0000x reference)
"""AlleleEmbedding v6: bf16 weight pipeline, q=64 layout, 512-slot chunks.

- Host casts kernel_table (+bias packed per half-row) to bf16: ktb2
  [RPC*2, 2112] bf16; halves gather DMA traffic and doubles PE moving rate.
- Chunk = 512 slots (8 g-blocks x 64 q). G tile [128, 8, 2112] bf16, gathered
  by 8 single-index indirect DMAs (partition p = dhalf*64+q, idx = 2*row+dhalf).
- a2 psum [128, 8*32] f32 via 16 small matmuls; copied to SBUF by ScalarE so
  both VectorE and GpSimd can read it (GpSimd cannot touch PSUM).
- Per t-range (8 of 32 t): TT mult (G-range x a2-broadcast -> P-range bf16)
  on VectorE or GpSimd (load-balanced), then 8 bf16 mask-matmuls (N = gmax*64
  <= 512) accumulating into psum out [64, 512] f32.
- Evacuation: VectorE TT add (psum + bias-bf16 from G cols 2048:2112) -> f32.
"""

import os
import numpy as np
import ml_dtypes

B, P, PLOIDY = 8, 5000, 2
NALLELES, NPOS, D = 16, 20000, 64
NCORES = 8
RPC = NPOS // NCORES

LAST_EXEC_TIME_NS = None
_NC_CACHE = {}

DEDUP = bool(int(os.environ.get("BASS_KERNEL_DEDUP", "1")))
# fraction knob: a unit goes to gpsimd when gp_load*RATIO < dve_load
GP_RATIO = float(os.environ.get("BASS_KERNEL_GP_RATIO", "2.3"))
TSPLIT = int(os.environ.get("BASS_KERNEL_TSPLIT", "8"))


def _build_nc(nchunks: int, units: tuple):
    """units: tuple of (chunk_id, gmax, on_gpsimd) in execution order."""
    import concourse.bass as bass
    import concourse.bacc as bacc
    import concourse.tile as tile
    from concourse import mybir

    f32 = mybir.dt.float32
    bf16 = mybir.dt.bfloat16
    nunits = len(units)
    nc = bacc.Bacc(None, target_bir_lowering=False, debug=False)
    kt2 = nc.declare_dram_parameter("kt2", [RPC * 2, 2112], bf16, isOutput=False)
    at = nc.declare_dram_parameter("at", [NALLELES, D], f32, isOutput=False)
    ct = nc.declare_dram_parameter("ct", [NALLELES, nunits * 512], f32, isOutput=False)
    mask = nc.declare_dram_parameter("mask", [128, 64], bf16, isOutput=False)
    idxg = nc.declare_dram_parameter("idxg", [nchunks, 8, 128, 1], mybir.dt.int32, isOutput=False)
    out = nc.declare_dram_parameter("out", [nunits, 64, 512], f32, isOutput=True)

    chunk_units = {}
    for u, (ck, gmax, on_gp) in enumerate(units):
        chunk_units.setdefault(ck, []).append((u, gmax, on_gp))

    with tile.TileContext(nc) as tc:
        with (
            tc.tile_pool(name="const", bufs=1) as cp,
            tc.tile_pool(name="g", bufs=2) as gp_pool,
            tc.tile_pool(name="p", bufs=4) as pp,
            tc.tile_pool(name="small", bufs=6) as sp,
            tc.tile_pool(name="psa", bufs=4, space="PSUM") as psa,
            tc.tile_pool(name="pso", bufs=4, space="PSUM") as pso,
        ):
            at_t = cp.tile([NALLELES, D], f32)
            nc.sync.dma_start(out=at_t[:], in_=at[:])
            mask_t = cp.tile([128, 64], bf16)
            nc.sync.dma_start(out=mask_t[:], in_=mask[:])
            ct_t = cp.tile([NALLELES, nunits * 512], f32)
            nc.sync.dma_start(out=ct_t[:], in_=ct[:])

            for ck in sorted(chunk_units.keys()):
                g_t = gp_pool.tile([128, 8, 2112], bf16, tag="g")
                for g in range(8):
                    ig_t = sp.tile([128, 1], mybir.dt.int32, tag="ig")
                    nc.sync.dma_start(out=ig_t[:], in_=idxg[ck, g])
                    nc.gpsimd.indirect_dma_start(
                        out=g_t[:, g], out_offset=None, in_=kt2[:],
                        in_offset=bass.IndirectOffsetOnAxis(ap=ig_t[:, :1], axis=0),
                    )
                for u, gmax, on_gp in chunk_units[ck]:
                    a2 = psa.tile([128, 256], f32, tag="a2")
                    for dhalf in range(2):
                        for g in range(gmax):
                            nc.tensor.matmul(
                                out=a2[dhalf * 64 : (dhalf + 1) * 64, g * 32 : (g + 1) * 32],
                                lhsT=ct_t[:, u * 512 + g * 64 : u * 512 + g * 64 + 64],
                                rhs=at_t[:, dhalf * 32 : (dhalf + 1) * 32],
                                start=True,
                                stop=True,
                                tile_position=(0, dhalf * 64),
                            )
                    a2s = sp.tile([128, 256], f32, tag="a2s")
                    nc.scalar.copy(out=a2s[:, : gmax * 32], in_=a2[:, : gmax * 32])
                    gv = g_t[:, :gmax, :2048].rearrange("p g (t e) -> p g t e", t=32)
                    a2v = (
                        a2s[:, : gmax * 32]
                        .rearrange("p (g t) -> p g t", g=gmax)
                        .unsqueeze(3)
                        .to_broadcast([128, gmax, 32, D])
                    )
                    eng = nc.gpsimd if on_gp else nc.vector
                    ops = pso.tile([64, 512], f32, tag="ops")
                    for k in range(0, 32, TSPLIT):
                        p_t = pp.tile([128, 8, TSPLIT, 64], bf16, tag="p")
                        eng.tensor_tensor(
                            out=p_t[:, :gmax],
                            in0=gv[:, :, k : k + TSPLIT],
                            in1=a2v[:, :, k : k + TSPLIT],
                            op=mybir.AluOpType.mult,
                        )
                        for t in range(TSPLIT):
                            nc.tensor.matmul(
                                out=ops[:, : gmax * 64],
                                lhsT=mask_t[:],
                                rhs=p_t[:, :gmax, t],
                                start=(k == 0 and t == 0),
                                stop=(k + t == 31),
                                skip_group_check=True,
                            )
                    ot = sp.tile([64, 512], f32, tag="ot")
                    nc.vector.tensor_tensor(
                        out=ot[:, : gmax * 64].rearrange("q (g e) -> q g e", g=gmax),
                        in0=ops[:, : gmax * 64].rearrange("q (g e) -> q g e", g=gmax),
                        in1=g_t[0:64, :gmax, 2048:2112],
                        op=mybir.AluOpType.add,
                    )
                    nc.sync.dma_start(out=out[u, :, : gmax * 64], in_=ot[:, : gmax * 64])
    nc.finalize()
    return nc


def _plan(local_rows: np.ndarray):
    n = len(local_rows)
    if DEDUP:
        rows_u, inv, counts_u = np.unique(
            local_rows, return_inverse=True, return_counts=True
        )
        ordr = np.argsort(-counts_u, kind="stable")
        rank_of = np.empty_like(ordr)
        rank_of[ordr] = np.arange(len(ordr))
        rank = rank_of[inv]
        row_by_rank = rows_u[ordr]
        count_by_rank = counts_u[ordr]
        order = np.argsort(rank, kind="stable")
        occ = np.empty(n, dtype=np.int64)
        cum = np.zeros(len(rows_u) + 1, dtype=np.int64)
        cum[1:] = np.cumsum(count_by_rank)
        occ[order] = np.arange(n) - cum[rank[order]]
        nslots = len(rows_u)
    else:
        rank = np.arange(n)
        occ = np.zeros(n, dtype=np.int64)
        row_by_rank = local_rows.astype(np.int64)
        count_by_rank = np.ones(n, dtype=np.int64)
        nslots = n

    nchunks = max(1, (nslots + 511) // 512)
    rows_p = np.zeros(nchunks * 512, dtype=np.int64)
    rows_p[:nslots] = row_by_rank
    counts_p = np.zeros(nchunks * 512, dtype=np.int64)
    counts_p[:nslots] = count_by_rank

    units = []  # (ck, j, gmax)
    for ck in range(nchunks):
        base = ck * 512
        npass = int(counts_p[base])
        for j in range(npass):
            width = int(np.count_nonzero(counts_p[base : base + 512] > j))
            gmax = (width + 63) // 64
            units.append((ck, j, gmax))
    unit_id_of = {(ck, j): i for i, (ck, j, _g) in enumerate(units)}
    pair_unit = np.array([unit_id_of[(r // 512, o)] for r, o in zip(rank, occ)])
    pair_slot = (rank % 512).astype(np.int64)

    return dict(
        nchunks=nchunks,
        units_full=units,
        rows_p=rows_p,
        pair_unit=pair_unit,
        pair_slot=pair_slot,
    )


def _gather_indices(plan, nchunks):
    rows_p = plan["rows_p"]
    own = plan["nchunks"]
    idxg = np.zeros((nchunks, 8, 128, 1), dtype=np.int32)
    for ck in range(own):
        rows_ck = rows_p[ck * 512 : (ck + 1) * 512]
        p = np.arange(128)
        for g in range(8):
            idxg[ck, g, :, 0] = 2 * rows_ck[g * 64 + (p % 64)] + p // 64
    return idxg


def kernel(alleles, positions, allele_table, kernel_table, bias_table):
    global LAST_EXEC_TIME_NS
    from concourse.bass_utils import run_bass_kernel_spmd

    alleles = np.asarray(alleles)
    positions = np.asarray(positions)
    allele_table = np.ascontiguousarray(np.asarray(allele_table), dtype=np.float32)
    kernel_table = np.ascontiguousarray(np.asarray(kernel_table), dtype=np.float32)
    bias_table = np.ascontiguousarray(np.asarray(bias_table), dtype=np.float32)

    pos = positions.reshape(-1).astype(np.int64)
    al = alleles.reshape(-1, PLOIDY)
    npairs = pos.shape[0]
    owner = pos // RPC
    local_row = pos % RPC
    cnt = (al[:, :, None] == np.arange(NALLELES)[None, None, :]).sum(1).astype(np.float32)

    mask_np = (np.arange(128)[:, None] % 64 == np.arange(64)[None, :]).astype(
        ml_dtypes.bfloat16
    )

    plans = []
    core_sel = []
    for c in range(NCORES):
        sel = np.where(owner == c)[0]
        core_sel.append(sel)
        plans.append(_plan(local_row[sel]))

    nchunks = max(p["nchunks"] for p in plans)
    pass_g = {}
    for p in plans:
        for ck, j, g in p["units_full"]:
            pass_g[(ck, j)] = max(pass_g.get((ck, j), 0), g)
    units_full = sorted(pass_g.keys())
    unit_id_of = {k: i for i, k in enumerate(units_full)}

    # greedy gpsimd/vector split balanced by gmax-weighted load
    dve_load, gp_load = 0.0, 0.0
    units = []
    for ck, j in units_full:
        g = pass_g[(ck, j)]
        if gp_load * GP_RATIO < dve_load and GP_RATIO > 0:
            units.append((ck, g, True))
            gp_load += g
        else:
            units.append((ck, g, False))
            dve_load += g
    units = tuple(units)
    nunits = len(units)

    key = (nchunks, units)
    if key not in _NC_CACHE:
        _NC_CACHE[key] = _build_nc(nchunks, units)
    nc = _NC_CACHE[key]

    in_maps = []
    pair_locs = []
    for c in range(NCORES):
        p = plans[c]
        remap = np.array([unit_id_of[(ck, j)] for ck, j, _g in p["units_full"]] or [0])
        pair_unit = remap[p["pair_unit"]]
        pair_locs.append((pair_unit, p["pair_slot"]))
        idxg = _gather_indices(p, nchunks)
        ct = np.zeros((NALLELES, nunits * 512), dtype=np.float32)
        sel = core_sel[c]
        ct[:, pair_unit * 512 + p["pair_slot"]] = cnt[sel].T

        ktb2 = np.zeros((RPC * 2, 2112), dtype=ml_dtypes.bfloat16)
        ktb2[:, :2048] = kernel_table[c * RPC : (c + 1) * RPC].reshape(RPC * 2, 2048)
        ktb2[0::2, 2048:] = bias_table[c * RPC : (c + 1) * RPC]
        in_maps.append(
            {
                "kt2": ktb2,
                "at": allele_table,
                "ct": ct,
                "mask": mask_np,
                "idxg": idxg,
            }
        )

    trace = bool(int(os.environ.get("BASS_KERNEL_TRACE", "0")))
    res = run_bass_kernel_spmd(nc, in_maps, core_ids=list(range(NCORES)), trace=trace)
    LAST_EXEC_TIME_NS = res.exec_time_ns

    out_full = np.zeros((npairs, D), dtype=np.float32)
    for c in range(NCORES):
        sel = core_sel[c]
        pair_unit, pair_slot = pair_locs[c]
        o = np.asarray(res.results[c]["out"])
        q = pair_slot % 64
        g = pair_slot // 64
        cols = (g * 64)[:, None] + np.arange(D)[None, :]
        out_full[sel] = o[pair_unit[:, None], q[:, None], cols]
    return out_full.reshape(B, P, D)

